# revision 23
# baseline (speedup 1.0000x reference)
"""Trainium2 Bass kernel for a dense transformer block (pre-LN, MHA + MLP).

Sharding: data-parallel over batch — B=8 batch elements, one per NeuronCore.
Each core runs the full block on its [1024, 768] slice; no collectives.

Design notes (v4):
  - Attention path runs in fp8e4 with DoubleRow matmuls (2 contraction rows
    per PE cell -> half the matmul count for qkv/AV/proj). Scales keep every
    fp8 tensor inside [subnormal, 240]: qkv_w x16, proj_w x8, exp bias -4,
    attn normalized to x32. End-to-end rel l2 err ~6e-3 on HW.
  - Scores (K=64 per head) run as row-tiled concurrent matmul pairs: head h
    of a pair occupies PE rows 64h..64h+63, so both heads' score matmuls for
    a key chunk execute simultaneously. Their two psum banks form one
    [P,2,NW] tile so a single ACT instruction exponentiates both heads.
  - MLP stays bf16 (fp8 there pushes rel err past the 2e-2 gate).
  - All weights are re-laid out on the HOST into their on-chip layouts so
    every weight DMA is one fully-contiguous-per-partition transfer (the
    strided version cost ~70us of descriptor storms at kernel start).
  - LN transposes run on the DMA XBAR; LN applies on DVE; softmax
    normalization: reciprocal_approx_fast on the [1,NW] denominator row +
    gpsimd partition_broadcast (PE and ACT never touch it).
  - Emission interleaves independent PE work into the exp-gated attention
    stream: qkv/v chunks under half-0's exps, proj/LN2/fc1(half0) under
    half-1's exps; fc1(half0) gelus are deferred (staged) so the ACT table
    never thrashes mid-stream.
"""

import os
import sys
from contextlib import ExitStack

import numpy as np
import ml_dtypes

for _p in ("/opt/trn_rl_repo",):
    if os.path.isdir(_p) and _p not in sys.path:
        sys.path.insert(0, _p)

import concourse.bass as bass  # noqa: E402
import concourse.mybir as mybir  # noqa: E402
import concourse.tile as tile  # noqa: E402
from concourse import bacc  # noqa: E402

B, SEQ, C, H, HD, HID = 8, 1024, 768, 12, 64, 3072
P = 128
FP = mybir.dt.float32
BF = mybir.dt.bfloat16
F8 = mybir.dt.float8e4
TC_N = SEQ // P          # 8 token chunks of 128
NW = 512                 # token-half width (matmul free dim)
KC = C // P              # 6 contraction chunks over C
KCP = KC // 2            # 3 DoubleRow chunk-pairs over C
HC_N = HID // P          # 24 hidden chunks
PAIRS = H // 2           # 6 head pairs
CS_W = C // 2            # 384-wide output slices for token-major matmuls
VSTR = 80                # per-head stride inside v_aug rows (16B-aligned ko step)
EPS = 1e-6

QKV_WS = 16.0            # host scale on qkv_w before fp8 cast
PW_S = 8.0               # host scale on proj_w before fp8 cast
ATT_S = 32.0             # scale carried by fp8 attn output
ONES_V = QKV_WS / ATT_S  # denominator column value in v_aug
EXP_B = -4.0             # exp(s - 4): keeps E in (0, ~47] for fp8
SCORE_SCALE = (HD ** -0.5) / (QKV_WS * QKV_WS)
PROJ_INV = 1.0 / (ATT_S * PW_S)

AF = mybir.ActivationFunctionType
OP = mybir.AluOpType
DR = mybir.MatmulPerfMode.DoubleRow


def _ln_to_fm(nc, tc, sc, x_big, tcx_list, out_fm, out_f8, pools, eps_t,
              g_t, b_t, apply_on_act=False):
    """LN over features of x_big[:, tcx, :] (token-major); write the
    transposed (feature-major) result via DMA XBAR transposes into a dense
    staging tile, then cast-copy into out_fm (fp8 if out_f8 else bf16).
    The rstd sqrt runs once for the whole tcx batch (one ACT table visit)."""
    v, s, dma = nc.vector, nc.scalar, nc.sync
    stat_pool, h_pool = pools
    n = len(tcx_list)
    mvs = stat_pool.tile([P, n, 2], FP, tag="mvs", name=f"mvs{sc}{tcx_list[0]}")
    for i, tcx in enumerate(tcx_list):
        xs = x_big[:, tcx, :]
        stats = stat_pool.tile([P, 3, 6], FP, tag="stats", name=f"st{sc}{tcx}")
        for j in range(3):
            v.bn_stats(out=stats[:, j, :], in_=xs[:, j * 256:(j + 1) * 256])
        v.bn_aggr(out=mvs[:, i, :], in_=stats)
    rstds = stat_pool.tile([P, n], FP, tag="rstds", name=f"rs{sc}{tcx_list[0]}")
    s.activation(out=rstds, in_=mvs[:, :, 1], func=AF.Sqrt, bias=eps_t,
                 scale=1.0)
    v.reciprocal(out=rstds, in_=rstds)
    for i, tcx in enumerate(tcx_list):
        xs = x_big[:, tcx, :]
        rstd = rstds[:, i:i + 1]
        nb = stat_pool.tile([P, 1], FP, tag="nb", name=f"nb{sc}{tcx}")
        v.tensor_scalar(out=nb, in0=mvs[:, i, 0:1], scalar1=rstd, scalar2=-1.0,
                        op0=OP.mult, op1=OP.mult)
        h_t = h_pool.tile([P, C], BF, tag="h_tm", name=f"htm{sc}{tcx}")
        with nc.allow_low_precision(reason="bf16 LN output"):
            if apply_on_act:
                s.activation(out=h_t, in_=xs, func=AF.Identity, bias=nb,
                             scale=rstd)
            else:
                v.tensor_scalar(out=h_t, in0=xs, scalar1=rstd, scalar2=nb,
                                op0=OP.mult, op1=OP.add)
            if g_t is not None:
                v.tensor_mul(out=h_t, in0=h_t, in1=g_t)
            if b_t is not None:
                v.tensor_add(out=h_t, in0=h_t, in1=b_t)
        stg = h_pool.tile([P, KC, P], BF, tag="h_stg", name=f"hs{sc}{tcx}")
        dma.dma_start_transpose(out=stg, in_=h_t)
        with nc.allow_low_precision(reason="matmul operand cast"):
            v.tensor_copy(out=out_fm[:, :, tcx * P:(tcx + 1) * P], in_=stg)


def _build(ln_affine: bool, proj_bias: bool):
    nc = bacc.Bacc("TRN2", debug=False)
    x_d = nc.dram_tensor("x", [SEQ, C], FP, kind="ExternalInput").ap()
    # all weights arrive host-re-laid so each DMA is contiguous per partition
    qkw_d = nc.dram_tensor("qk_w8", [P, 2 * KC, KC, P], F8,
                           kind="ExternalInput").ap()
    vw_d = nc.dram_tensor("v_w8", [P, KC, C], F8, kind="ExternalInput").ap()
    pw_d = nc.dram_tensor("proj_w8", [P, KC, C], F8, kind="ExternalInput").ap()
    fc1w_d = nc.dram_tensor("fc1_wb", [P, KC, HID], BF,
                            kind="ExternalInput").ap()
    fc2w_d = nc.dram_tensor("fc2_wb", [P, HC_N, C], BF,
                            kind="ExternalInput").ap()
    fc1b_d = nc.dram_tensor("fc1_br", [P, HC_N], FP, kind="ExternalInput").ap()
    lnp = {}
    if ln_affine:
        for nm in ("ln1_g", "ln1_b", "ln2_g", "ln2_b"):
            lnp[nm] = nc.dram_tensor(nm, [C], FP, kind="ExternalInput").ap()
    if proj_bias:
        lnp["proj_b"] = nc.dram_tensor("proj_b", [C], FP, kind="ExternalInput").ap()
    out_d = nc.dram_tensor("out", [SEQ, C], FP, kind="ExternalOutput").ap()

    with tile.TileContext(nc) as tc:
        with ExitStack() as ctx:
            _body(nc, tc, ctx, x_d, qkw_d, vw_d, pw_d, fc1w_d, fc2w_d, fc1b_d,
                  lnp, out_d, ln_affine, proj_bias)
    nc.compile()
    return nc


def _body(nc, tc, ctx, x_d, qkw_d, vw_d, pw_d, fc1w_d, fc2w_d, fc1b_d, lnp,
          out_d, ln_affine, proj_bias):
    v, s, te, dma = nc.vector, nc.scalar, nc.tensor, nc.sync
    lp = nc.allow_low_precision

    # ---------- persistent pool ----------
    p0 = ctx.enter_context(tc.tile_pool(name="p0", bufs=1))
    x_tm = p0.tile([P, TC_N, C], FP)       # holds x, then x1, then out
    x_src = x_d.rearrange("(tc p) c -> p tc c", p=P)
    for tcx in range(TC_N):
        eng = dma if tcx < 4 else nc.scalar
        eng.dma_start(out=x_tm[:, tcx, :], in_=x_src[:, tcx, :])
    eps_t = p0.tile([P, 1], FP)
    v.memset(eps_t, EPS)
    expb_t = p0.tile([P, 1], FP)
    v.memset(expb_t, EXP_B)
    fc1b_t = p0.tile([P, HC_N], FP)
    nc.scalar.dma_start(out=fc1b_t, in_=fc1b_d)
    warm_a = p0.tile([P, P], BF)
    v.memset(warm_a, 0.0)
    warm_b = p0.tile([P, 256], BF)
    v.memset(warm_b, 0.0)

    def bcast_c(pool, name):
        if name not in lnp:
            return None
        t = pool.tile([P, C], FP, name=name + "_bc", tag=name, bufs=1)
        src = lnp[name]
        ap = bass.AP(tensor=src.tensor, offset=src.offset, ap=[[0, P], src.ap[0]])
        nc.gpsimd.dma_start(out=t, in_=ap)
        return t

    # ---------- resident weights (one contiguous DMA each) ----------
    wts = ctx.enter_context(tc.tile_pool(name="wts", bufs=1))
    pw_buf = wts.tile([P, KC, C], F8)
    nc.scalar.dma_start(out=pw_buf, in_=pw_d)
    w1_buf = wts.tile([P, KC, HID], BF)
    nc.scalar.dma_start(out=w1_buf, in_=fc1w_d)

    # attention-persistent tensors
    ap0 = ctx.enter_context(tc.tile_pool(name="ap0", bufs=1))
    q_fm = ap0.tile([P, PAIRS, SEQ], F8)   # pair-packed: head 2p on parts 0-63
    k_fm = ap0.tile([P, PAIRS, SEQ], F8)
    v_aug = ap0.tile([P, TC_N, H * VSTR], F8)  # per head: 64 v cols + denom col
    attn_fm = ap0.tile([P, KC, SEQ], F8)
    h2_fm = ap0.tile([P, KC, SEQ], BF)
    g_fm = ap0.tile([P, HC_N, SEQ], BF)
    va4 = v_aug.rearrange("p tc (h e) -> p tc h e", e=VSTR)
    v.memset(va4[:, :, :, 64:65], ONES_V)

    out_r = out_d.rearrange("(tc p) c -> p tc c", p=P)

    # ---------- pools ----------
    stat_pool = ctx.enter_context(tc.tile_pool(name="st", bufs=4))
    h_pool = ctx.enter_context(tc.tile_pool(name="htm", bufs=2))
    att = ctx.enter_context(tc.tile_pool(name="att", bufs=1))
    e_pool = ctx.enter_context(tc.tile_pool(name="epool", bufs=2))
    # 8 psum banks: scores 2x2 (two-bank [P,2,NW] tiles), U-accum 2 (1 per
    # head tag), shared qkv/proj/fc1/fc2 pool 2 (live ranges time-disjoint).
    sps = ctx.enter_context(tc.tile_pool(name="sps", bufs=2, space="PSUM"))
    ups = ctx.enter_context(tc.tile_pool(name="ups", bufs=1, space="PSUM"))
    cps = ctx.enter_context(tc.tile_pool(name="cps", bufs=2, space="PSUM"))
    ln2_g = bcast_c(att, "ln2_g")
    ln2_b = bcast_c(att, "ln2_b")
    projb_t = None
    if proj_bias:
        projb_t = att.tile([P, C], FP, name="projb_s", tag="projb_s", bufs=1)
        pb = bcast_c(att, "proj_b")
        v.tensor_scalar(out=projb_t, in0=pb, scalar1=1.0 / PROJ_INV,
                        scalar2=0.0, op0=OP.mult, op1=OP.add)

    # ---------- LN1 (scoped; freed after half-0 attention) ----------
    s1 = ExitStack()
    s1p = s1.enter_context(tc.tile_pool(name="s1", bufs=1))
    h1_f8 = s1p.tile([P, KC, SEQ], F8)
    qk_buf = s1p.tile([P, 2 * KC, KC, P], F8)
    nc.scalar.dma_start(out=qk_buf, in_=qkw_d)
    vw_buf = s1p.tile([P, KC, C], F8)
    nc.scalar.dma_start(out=vw_buf, in_=vw_d)
    ln1_g = bcast_c(s1p, "ln1_g")
    ln1_b = bcast_c(s1p, "ln1_b")
    _ln_to_fm(nc, tc, 1, x_tm, [0, 1, 2, 3], h1_f8, True,
              (stat_pool, h_pool), eps_t, ln1_g, ln1_b, apply_on_act=True)

    # PE warmup while LN1 runs on ACT/DVE/DMA
    for i in range(12):
        wp = cps.tile([P, 256], FP, tag="cps", name=f"warm{i}")
        te.matmul(wp, lhsT=warm_a, rhs=warm_b, start=True, stop=True)
    _ln_to_fm(nc, tc, 1, x_tm, [4, 5, 6, 7], h1_f8, True,
              (stat_pool, h_pool), eps_t, ln1_g, ln1_b, apply_on_act=True)

    # ---------- emission units ----------
    def qk_half(f, nn):
        """f in 0..5 -> q pair f; 6..11 -> k pair f-6; one token half."""
        nsl = slice(nn * NW, (nn + 1) * NW)
        ps = cps.tile([P, NW], FP, tag="cps", name=f"qkp{f}{nn}")
        for kcp in range(KCP):
            te.matmul(ps, lhsT=qk_buf[:, f, 2 * kcp:2 * kcp + 2, :],
                      rhs=h1_f8[:, 2 * kcp:2 * kcp + 2, nsl],
                      start=kcp == 0, stop=kcp == KCP - 1, perf_mode=DR)
        dst = q_fm if f < PAIRS else k_fm
        with lp(reason="fp8 matmul operand"):
            v.tensor_copy(out=dst[:, f % PAIRS, nsl], in_=ps)

    def v_chunk(tcx, vs):
        ps = cps.tile([P, CS_W], FP, tag="cps", name=f"vp{tcx}{vs}")
        for kcp in range(KCP):
            te.matmul(ps, lhsT=h1_f8[:, 2 * kcp:2 * kcp + 2,
                                     tcx * P:(tcx + 1) * P],
                      rhs=vw_buf[:, 2 * kcp:2 * kcp + 2,
                                 vs * CS_W:(vs + 1) * CS_W],
                      start=kcp == 0, stop=kcp == KCP - 1, perf_mode=DR)
        with lp(reason="fp8 matmul operand"):
            v.tensor_copy(out=va4[:, tcx, 6 * vs:6 * vs + 6, 0:64], in_=ps)

    def proj_unit(tcx, cs):
        ps = cps.tile([P, CS_W], FP, tag="cps", name=f"pp{tcx}{cs}")
        for kcp in range(KCP):
            te.matmul(ps, lhsT=attn_fm[:, 2 * kcp:2 * kcp + 2,
                                       tcx * P:(tcx + 1) * P],
                      rhs=pw_buf[:, 2 * kcp:2 * kcp + 2,
                                 cs * CS_W:(cs + 1) * CS_W],
                      start=kcp == 0, stop=kcp == KCP - 1, perf_mode=DR)
        xsl = x_tm[:, tcx, cs * CS_W:(cs + 1) * CS_W]
        if projb_t is not None:
            v.tensor_add(out=ps, in0=ps,
                         in1=projb_t[:, cs * CS_W:(cs + 1) * CS_W])
        v.scalar_tensor_tensor(out=xsl, in0=ps, scalar=PROJ_INV,
                               in1=xsl, op0=OP.mult, op1=OP.add)

    def ln2_half(nn):
        _ln_to_fm(nc, tc, 2, x_tm, list(range(nn * 4, nn * 4 + 4)), h2_fm,
                  False, (stat_pool, h_pool), eps_t, ln2_g, ln2_b)

    def fc1_chunk(hc, nn, staged):
        nsl = slice(nn * NW, (nn + 1) * NW)
        ps = cps.tile([P, NW], FP, tag="cps", name=f"f1p{hc}{nn}")
        for kc in range(KC):
            te.matmul(ps, lhsT=w1_buf[:, kc, hc * P:(hc + 1) * P],
                      rhs=h2_fm[:, kc, nsl],
                      start=kc == 0, stop=kc == KC - 1)
        with lp(reason="bf16 mlp activations"):
            if staged:
                # stage pre-gelu in bf16; gelu applied in-place later so the
                # ACT exp stream isn't broken by table switches.
                v.tensor_copy(out=g_fm[:, hc, nsl], in_=ps)
            else:
                s.activation(out=g_fm[:, hc, nsl], in_=ps, func=AF.Gelu,
                             bias=fc1b_t[:, hc:hc + 1], scale=1.0)

    def gelu_chunk(hc, nn):
        nsl = slice(nn * NW, (nn + 1) * NW)
        with lp(reason="bf16 mlp activations"):
            s.activation(out=g_fm[:, hc, nsl], in_=g_fm[:, hc, nsl],
                         func=AF.Gelu, bias=fc1b_t[:, hc:hc + 1], scale=1.0)

    def fc2_grp(grp, nn, store):
        for tcx in range(nn * 4, nn * 4 + 4):
            pss = [cps.tile([P, CS_W], FP, tag="cps",
                            name=f"f2p{nn}{grp}{tcx}{cs}") for cs in range(2)]
            for i, hc in enumerate(range(grp * 6, grp * 6 + 6)):
                for cs in range(2):
                    te.matmul(pss[cs],
                              lhsT=g_fm[:, hc, tcx * P:(tcx + 1) * P],
                              rhs=w2_buf[:, hc, cs * CS_W:(cs + 1) * CS_W],
                              start=i == 0, stop=i == 5)
            for cs in range(2):
                xsl = x_tm[:, tcx, cs * CS_W:(cs + 1) * CS_W]
                v.tensor_add(out=xsl, in0=pss[cs], in1=xsl)
            if store:
                dma.dma_start(out=out_r[:, tcx, :], in_=x_tm[:, tcx, :])

    # ---------- attention with PE filler interleave ----------
    def attend_pair(pr, nn, fillers):
        """scores (row-tiled head pair) + merged exp + DoubleRow AV +
        broadcast normalize. `fillers[step]` emits independent PE work
        between the scores and the exp-gated AV matmuls."""
        nsl = slice(nn * NW, (nn + 1) * NW)
        psU = {hh: ups.tile([65, NW], FP, tag=f"u{hh}", name=f"u{pr}{nn}{hh}")
               for hh in (0, 1)}
        ets = {}
        for step in range(5):            # 4 mc-pairs + 1 drain; AV skewed by 1
            if step < 4:
                cur = e_pool.tile([P, 2, 2, NW], F8, tag="E",
                                  name=f"E{pr}{nn}{step}")
                for mcs in range(2):
                    mc = 2 * step + mcs
                    sp2 = sps.tile([P, 2, NW], FP, tag="sps",
                                   name=f"sc{pr}{nn}{mc}")
                    for hh in (0, 1):
                        hsl = slice(64 * hh, 64 * hh + 64)
                        te.matmul(sp2[:, hh, :],
                                  lhsT=k_fm[hsl, pr, mc * P:(mc + 1) * P],
                                  rhs=q_fm[hsl, pr, nsl], start=True, stop=True)
                    with lp(reason="fp8 softmax weights"):
                        s.activation(out=cur[:, :, mcs, :], in_=sp2,
                                     func=AF.Exp, scale=SCORE_SCALE,
                                     bias=expb_t)
                ets[step] = cur
            for fill in fillers.get(step, ()):
                fill()
            if step > 0:
                j = step - 1
                prev = ets.pop(j)
                for hh in (0, 1):
                    ha = 2 * pr + hh
                    te.matmul(psU[hh],
                              lhsT=v_aug[:, 2 * j:2 * j + 2,
                                         ha * VSTR:ha * VSTR + 65],
                              rhs=prev[:, hh, :, :], start=j == 0, stop=j == 3,
                              perf_mode=DR)
        for hh in (0, 1):
            srow = att.tile([1, NW], FP, tag="sr", name=f"sr{pr}{nn}{hh}",
                            bufs=1)
            rrow = att.tile([1, NW], FP, tag="rr", name=f"rr{pr}{nn}{hh}",
                            bufs=1)
            rt_sb = att.tile([64, NW], FP, tag="rt",
                             name=f"rt{pr}{nn}{hh}", bufs=2)
            v.tensor_copy(out=srow, in_=psU[hh][64:65, :])
            v.reciprocal_approx_fast(out=rrow, in_=srow)
            nc.gpsimd.partition_broadcast(rt_sb, rrow)
            with lp(reason="fp8 matmul operand"):
                v.tensor_mul(out=attn_fm[64 * hh:64 * hh + 64, pr, nsl],
                             in0=psU[hh][0:64, :], in1=rt_sb)

    # -- token half 0: attention, qkv chunks as PE filler --
    # Each pair's q/k chunks and the v chunks are produced at least one
    # attend-step before their first reader (scores of the NEXT pair / the
    # AV matmul of the current step).
    qk_half(6, 0), qk_half(6, 1), qk_half(0, 0), qk_half(0, 1)
    F = lambda fn, *a: (lambda: fn(*a))
    fill_nn0 = [
        {0: [F(v_chunk, 0, 0), F(v_chunk, 1, 0)],
         1: [F(v_chunk, 2, 0), F(qk_half, 7, 0)],
         2: [F(v_chunk, 3, 0), F(v_chunk, 4, 0), F(qk_half, 7, 1)],
         3: [F(v_chunk, 5, 0), F(qk_half, 1, 0)],
         4: [F(v_chunk, 6, 0), F(v_chunk, 7, 0), F(qk_half, 1, 1)]},
        {0: [F(v_chunk, 0, 1)], 1: [F(qk_half, 8, 0), F(v_chunk, 1, 1)],
         2: [F(qk_half, 8, 1)], 3: [F(qk_half, 2, 0)], 4: [F(qk_half, 2, 1)]},
        {0: [F(v_chunk, 2, 1)], 1: [F(qk_half, 9, 0), F(v_chunk, 3, 1)],
         2: [F(qk_half, 9, 1)], 3: [F(qk_half, 3, 0)], 4: [F(qk_half, 3, 1)]},
        {0: [F(v_chunk, 4, 1)], 1: [F(qk_half, 10, 0), F(v_chunk, 5, 1)],
         2: [F(qk_half, 10, 1), F(v_chunk, 6, 1)],
         3: [F(qk_half, 4, 0), F(v_chunk, 7, 1)], 4: [F(qk_half, 4, 1)]},
        {0: [F(qk_half, 11, 0)], 1: [F(qk_half, 11, 1)],
         2: [F(qk_half, 5, 0)], 3: [F(qk_half, 5, 1)]},
        {},
    ]
    for pr in range(PAIRS):
        attend_pair(pr, 0, fill_nn0[pr])
    s1.close()   # frees h1 sbuf before the fc2 weights land
    w2p = ctx.enter_context(tc.tile_pool(name="w2", bufs=1))
    w2_buf = w2p.tile([P, HC_N, C], BF)
    nc.scalar.dma_start(out=w2_buf, in_=fc2w_d)

    # -- token half 1 attention; fillers: proj+LN2(half0) then fc1(half0) --
    fill_nn1 = [dict() for _ in range(PAIRS)]
    fill_nn1[0] = {0: [F(proj_unit, 0, 0), F(proj_unit, 0, 1)],
                   1: [F(proj_unit, 1, 0), F(proj_unit, 1, 1)],
                   2: [F(proj_unit, 2, 0), F(proj_unit, 2, 1)],
                   3: [F(proj_unit, 3, 0), F(proj_unit, 3, 1)],
                   4: [F(ln2_half, 0)]}
    hc_iter = iter(range(HC_N))
    for pr in range(1, PAIRS):
        for step in range(5):
            take = [next(hc_iter, None) for _ in range(1 if step < 4 else 2)]
            fill_nn1[pr][step] = [F(fc1_chunk, hc, 0, True)
                                  for hc in take if hc is not None]
    for pr in range(PAIRS):
        attend_pair(pr, 1, fill_nn1[pr])
    for hc in hc_iter:
        fc1_chunk(hc, 0, staged=True)

    # -- tail: proj+LN2(half1), gelu(half0) batch, fc2(0), mlp(1) --
    for tcx in range(4, 8):
        proj_unit(tcx, 0), proj_unit(tcx, 1)
    ln2_half(1)
    for grp in range(4):
        for hc in range(grp * 6, grp * 6 + 6):
            gelu_chunk(hc, 0)
        fc2_grp(grp, 0, store=grp == 3)
        for hc in range(grp * 6, grp * 6 + 6):
            fc1_chunk(hc, 1, staged=False)
    for grp in range(4):
        fc2_grp(grp, 1, store=grp == 3)


_CACHE = {}
last_results = None


def _get_nc(ln_affine, proj_bias):
    key = (ln_affine, proj_bias)
    if key not in _CACHE:
        _CACHE[key] = _build(*key)
    return _CACHE[key]


def kernel(x, qkv_w, proj_w, proj_b, ln1_g, ln1_b, ln2_g, ln2_b,
           fc1_w, fc1_b, fc2_w, fc2_b):
    global last_results
    from concourse.bass_utils import run_bass_kernel_spmd

    f32 = lambda a: np.ascontiguousarray(np.asarray(a), dtype=np.float32)
    x, qkv_w, proj_w, fc1_w, fc2_w = map(f32, (x, qkv_w, proj_w, fc1_w, fc2_w))
    proj_b, fc1_b, fc2_b = map(f32, (proj_b, fc1_b, fc2_b))
    ln1_g, ln1_b, ln2_g, ln2_b = map(f32, (ln1_g, ln1_b, ln2_g, ln2_b))

    ln_affine = not (np.all(ln1_g == 1) and np.all(ln1_b == 0)
                     and np.all(ln2_g == 1) and np.all(ln2_b == 0))
    proj_bias = bool(np.any(proj_b != 0))
    nc = _get_nc(ln_affine, proj_bias)

    fp8 = lambda a: np.ascontiguousarray(
        np.clip(a, -240.0, 240.0).astype(ml_dtypes.float8_e4m3))
    bf = lambda a: np.ascontiguousarray(a.astype(ml_dtypes.bfloat16))
    # host re-layouts: [kc*128+p, f] -> [p, ..., f] so DMAs are contiguous
    qk = (qkv_w[:, :2 * C] * QKV_WS).reshape(KC, P, 2 * KC, P)
    qk = qk.transpose(1, 2, 0, 3)                        # [p, f, kc, 128]
    vw = (qkv_w[:, 2 * C:] * QKV_WS).reshape(KC, P, C).transpose(1, 0, 2)
    pw = (proj_w * PW_S).reshape(KC, P, C).transpose(1, 0, 2)
    w1 = fc1_w.reshape(KC, P, HID).transpose(1, 0, 2)
    w2 = fc2_w.reshape(HC_N, P, C).transpose(1, 0, 2)
    f1b = fc1_b.reshape(HC_N, P).T                        # [p, hc]

    common = {"qk_w8": fp8(qk), "v_w8": fp8(vw), "proj_w8": fp8(pw),
              "fc1_wb": bf(w1), "fc2_wb": bf(w2),
              "fc1_br": np.ascontiguousarray(f1b)}
    if ln_affine:
        common.update({"ln1_g": ln1_g, "ln1_b": ln1_b,
                       "ln2_g": ln2_g, "ln2_b": ln2_b})
    if proj_bias:
        common["proj_b"] = proj_b
    in_maps = [dict(common, x=np.ascontiguousarray(x[b])) for b in range(B)]

    res = run_bass_kernel_spmd(nc, in_maps, core_ids=list(range(B)))
    last_results = res
    out = np.stack([r["out"] for r in res.results], axis=0)
    # fc2_b commutes past the final residual add — fold on host.
    return (out + fc2_b[None, None, :]).astype(np.float32)
